# revision 5
# baseline (speedup 1.0000x reference)
"""Trainium2 Bass kernel for the BottleneckBlock (conv -> IN -> lrelu -> self-attn
-> conv -> IN -> +residual -> lrelu), data-parallel over batch across 8 cores.

Layout per core (one batch element): channels on partitions, length L on the
free dimension.  All matmuls run in bf16 (fp32 PSUM accumulation); softmax Z is
reduced over partitions with a ones-column matmul and broadcast back with a
K=1 fp32 matmul.  v is produced directly transposed (vT = hT @ wvT) so the
P@V contraction needs no PE transposes.  b1/b2 are dropped: InstanceNorm
cancels any per-channel constant bias.

All weights/biases are packed into a single [128, F] DRAM tensor (one DMA, one
completion semaphore) and x into another — the walrus backend allows only a
couple of sync-waits per instruction, so input loads must not scatter across
many DMA-completion lanes.
"""
import numpy as np
import ml_dtypes

import concourse.bass as bass
import concourse.bacc as bacc
import concourse.mybir as mybir
import concourse.tile as tile
from concourse.bass_utils import run_bass_kernel_spmd

DT = mybir.dt
ALU = mybir.AluOpType
AF = mybir.ActivationFunctionType
BF16 = ml_dtypes.bfloat16

B, C, L = 8, 256, 2048
CR, CO, KW = 32, 512, 5
PAD = KW // 2
LP = L + 2 * PAD          # padded length
NCH = L // 512            # 512-wide l-chunks
NMT = L // 128            # 128-wide m-tiles
SCALE = CR ** (-0.5)
EPS = 1e-5
SLOPE = 0.2

# packed-weights segment offsets (elements per partition, bf16)
_SEG = {}
_off = 0
for _name, _sz in (("w1t", 2 * KW * C), ("w2t", 2 * KW * CO),
                   ("wqt", 2 * CR), ("wkt", 2 * CR), ("wvt", 2 * C),
                   ("wot", 2 * C), ("wrt", 2 * CO), ("ones_col", 1)):
    _SEG[_name] = (_off, _off + _sz)
    _off += _sz
# row-0 segments (biases + ones row)
for _name, _sz in (("bq", CR), ("bk", CR), ("bv", C), ("bo", C), ("br", CO),
                   ("ones_row", 512)):
    _SEG[_name] = (_off, _off + _sz)
    _off += _sz
F_PACK = _off

_CACHED_NC = None


def _build():
    nc = bacc.Bacc("TRN2", target_bir_lowering=False)

    x_d = nc.dram_tensor("x", [128, 2, LP], DT.bfloat16, kind="ExternalInput")
    wp_d = nc.dram_tensor("wpack", [128, F_PACK], DT.bfloat16, kind="ExternalInput")
    out_d = nc.dram_tensor("out", [CO, L], DT.float32, kind="ExternalOutput")

    with tile.TileContext(nc) as tc:
        with (
            tc.tile_pool(name="consts", bufs=1) as consts,
            tc.tile_pool(name="big", bufs=1) as big,
            tc.tile_pool(name="raw", bufs=2) as rawp,
            tc.tile_pool(name="ptp", bufs=2) as ptp,
            tc.tile_pool(name="stat", bufs=2) as statp,
            tc.tile_pool(name="small", bufs=8) as smallp,
            tc.tile_pool(name="tmp", bufs=4) as tmpp,
            tc.tile_pool(name="outp", bufs=4) as outp,
            tc.tile_pool(name="psw", bufs=4, space="PSUM") as psw,
            tc.tile_pool(name="psacc", bufs=1, space="PSUM") as psacc,
            tc.tile_pool(name="psz", bufs=1, space="PSUM") as psz,
        ):
            wall = consts.tile([128, F_PACK], DT.bfloat16, tag="wall")
            nc.sync.dma_start(out=wall, in_=wp_d[:, :])

            def seg(name):
                a, b = _SEG[name]
                return wall[:, a:b]

            w1t = seg("w1t").rearrange("p (i k o) -> p i k o", i=2, k=KW)
            w2t = seg("w2t").rearrange("p (i k o) -> p i k o", i=2, k=KW)
            wqt = seg("wqt").rearrange("p (i o) -> p i o", i=2)
            wkt = seg("wkt").rearrange("p (i o) -> p i o", i=2)
            wvt = seg("wvt").rearrange("p (i o) -> p i o", i=2)
            wot = seg("wot").rearrange("p (i o) -> p i o", i=2)
            wrt = seg("wrt").rearrange("p (i o) -> p i o", i=2)
            ones_col = seg("ones_col")
            bq = seg("bq")[0:1]
            bk = seg("bk")[0:1]
            bv = seg("bv")[0:1]
            bo = seg("bo")[0:1]
            br = seg("br")[0:1]
            ones_row = seg("ones_row")[0:1]
            ones_bf = ones_row[:, 0:128]

            ones_f32 = consts.tile([1, 128], DT.float32, tag="ones_f32")
            nc.vector.tensor_copy(ones_f32, ones_bf)
            eps_t = consts.tile([128, 1], DT.float32, tag="eps")
            nc.vector.memset(eps_t, EPS)

            # ---------------- persistent activations ----------------
            xall = big.tile([128, 2, LP], DT.bfloat16, tag="xall")
            nc.sync.dma_start(out=xall, in_=x_d[:, :, :])
            xp = [xall[:, i, :] for i in range(2)]
            hp = [big.tile([128, LP], DT.bfloat16, tag=f"hp{i}", name=f"hp{i}")
                  for i in range(2)]
            h2p = [big.tile([128, LP], DT.bfloat16, tag=f"h2p{i}", name=f"h2p{i}")
                   for i in range(2)]
            for i in range(2):
                for t in (hp[i], h2p[i]):
                    nc.vector.memset(t[:, 0:PAD], 0.0)
                    nc.vector.memset(t[:, LP - PAD:LP], 0.0)
            qs = big.tile([32, L], DT.bfloat16, tag="qs")
            ks = big.tile([32, L], DT.bfloat16, tag="ks")
            vT = big.tile([128, NMT, C], DT.bfloat16, tag="vT")
            os_ = [big.tile([128, L], DT.bfloat16, tag=f"os{i}", name=f"os{i}")
                   for i in range(2)]

            def mm(p, lhsT, rhs, first, last):
                nc.tensor.matmul(p, lhsT=lhsT, rhs=rhs, start=first, stop=last)

            # ---------------- conv1 + instance norm + leaky ----------------
            for t in range(2):
                osl = slice(t * 128, (t + 1) * 128)
                raw = rawp.tile([128, L], DT.float32, tag="h1raw")
                st = statp.tile([128, NCH, 6], DT.float32, tag="st1")
                for lc in range(NCH):
                    p = psw.tile([128, 512], DT.float32, tag="w")
                    n = 0
                    for i in range(2):
                        for k in range(KW):
                            mm(p, w1t[:, i, k, osl],
                               xp[i][:, lc * 512 + k: lc * 512 + k + 512],
                               n == 0, n == 9)
                            n += 1
                    nc.vector.bn_stats(out=st[:, lc, :], in_=p)
                    nc.scalar.copy(out=raw[:, lc * 512:(lc + 1) * 512], in_=p)
                mv = smallp.tile([128, 2], DT.float32, tag="mv")
                rstd = smallp.tile([128, 1], DT.float32, tag="rstd")
                negm = smallp.tile([128, 1], DT.float32, tag="negm")
                nc.vector.bn_aggr(out=mv, in_=st)
                nc.scalar.activation(out=rstd, in_=mv[:, 1:2], func=AF.Sqrt,
                                     bias=eps_t, scale=1.0)
                nc.vector.reciprocal(out=rstd, in_=rstd)
                nc.vector.tensor_scalar(out=negm, in0=mv[:, 0:1], scalar1=rstd,
                                        scalar2=-1.0, op0=ALU.mult, op1=ALU.mult)
                for lc in range(NCH):
                    tmp = tmpp.tile([128, 512], DT.float32, tag="tmp")
                    nc.scalar.activation(out=tmp, in_=raw[:, lc * 512:(lc + 1) * 512],
                                         func=AF.Identity, bias=negm, scale=rstd)
                    nc.vector.scalar_tensor_tensor(
                        out=hp[t][:, PAD + lc * 512:PAD + (lc + 1) * 512],
                        in0=tmp, scalar=SLOPE, in1=tmp, op0=ALU.mult, op1=ALU.max)

            # ---------------- q, k (1x1 convs on h) ----------------
            for dst, wt, bias in ((qs, wqt, bq), (ks, wkt, bk)):
                for lc in range(NCH):
                    lsl = slice(PAD + lc * 512, PAD + lc * 512 + 512)
                    p = psw.tile([32, 512], DT.float32, tag="w")
                    mm(p, wt[:, 0, :], hp[0][:, lsl], True, False)
                    mm(p, wt[:, 1, :], hp[1][:, lsl], False, False)
                    mm(p, bias, ones_row, False, True)
                    nc.scalar.copy(out=dst[:, lc * 512:(lc + 1) * 512], in_=p)

            # ---------------- vT[m, c] = h[:, m].T @ wvT + bv ----------------
            for mt in range(NMT):
                msl = slice(PAD + mt * 128, PAD + mt * 128 + 128)
                p = psw.tile([128, C], DT.float32, tag="w")
                mm(p, hp[0][:, msl], wvt[:, 0, :], True, False)
                mm(p, hp[1][:, msl], wvt[:, 1, :], False, False)
                mm(p, ones_bf, bv, False, True)
                nc.scalar.copy(out=vT[:, mt, :], in_=p)

            # ---------------- attention per l-chunk ----------------
            for lc in range(NCH):
                lsl = slice(lc * 512, (lc + 1) * 512)
                pt = ptp.tile([128, NMT, 512], DT.bfloat16, tag="pt")
                po = [psacc.tile([128, 512], DT.float32, tag=f"oc{t}", name=f"oc{t}")
                      for t in range(2)]
                pz = psz.tile([1, 512], DT.float32, tag="z")
                for mt in range(NMT):
                    ps = psw.tile([128, 512], DT.float32, tag="w")
                    mm(ps, ks[:, mt * 128:(mt + 1) * 128], qs[:, lsl], True, True)
                    nc.scalar.activation(out=pt[:, mt, :], in_=ps, func=AF.Exp,
                                         scale=SCALE)
                    for t in range(2):
                        mm(po[t], vT[:, mt, t * 128:(t + 1) * 128], pt[:, mt, :],
                           mt == 0, mt == NMT - 1)
                    mm(pz, ones_col, pt[:, mt, :], mt == 0, mt == NMT - 1)
                zrec = smallp.tile([1, 512], DT.float32, tag="zrec")
                nc.vector.reciprocal(out=zrec, in_=pz)
                pbc = psw.tile([128, 512], DT.float32, tag="w")
                mm(pbc, ones_f32, zrec, True, True)
                bcs = tmpp.tile([128, 512], DT.float32, tag="bcs")
                nc.scalar.copy(out=bcs, in_=pbc)
                for t in range(2):
                    nc.vector.tensor_tensor(out=os_[t][:, lsl], in0=po[t], in1=bcs,
                                            op=ALU.mult)

            # ---------------- h2 = wo @ o + bo + h ----------------
            for t in range(2):
                osl = slice(t * 128, (t + 1) * 128)
                for lc in range(NCH):
                    lsl = slice(lc * 512, (lc + 1) * 512)
                    p = psw.tile([128, 512], DT.float32, tag="w")
                    mm(p, wot[:, 0, osl], os_[0][:, lsl], True, False)
                    mm(p, wot[:, 1, osl], os_[1][:, lsl], False, False)
                    mm(p, bo[:, osl], ones_row, False, True)
                    nc.vector.tensor_tensor(
                        out=h2p[t][:, PAD + lc * 512:PAD + (lc + 1) * 512],
                        in0=p, in1=hp[t][:, PAD + lc * 512:PAD + (lc + 1) * 512],
                        op=ALU.add)

            # ---------- conv2 + IN, residual conv on x, leaky, store ----------
            for t in range(4):
                osl = slice(t * 128, (t + 1) * 128)
                raw = rawp.tile([128, L], DT.float32, tag="c2raw")
                st = statp.tile([128, NCH, 6], DT.float32, tag="st2")
                for lc in range(NCH):
                    p = psw.tile([128, 512], DT.float32, tag="w")
                    n = 0
                    for i in range(2):
                        for k in range(KW):
                            mm(p, w2t[:, i, k, osl],
                               h2p[i][:, lc * 512 + k: lc * 512 + k + 512],
                               n == 0, n == 9)
                            n += 1
                    nc.vector.bn_stats(out=st[:, lc, :], in_=p)
                    nc.scalar.copy(out=raw[:, lc * 512:(lc + 1) * 512], in_=p)
                mv = smallp.tile([128, 2], DT.float32, tag="mv")
                rstd = smallp.tile([128, 1], DT.float32, tag="rstd")
                negm = smallp.tile([128, 1], DT.float32, tag="negm")
                nc.vector.bn_aggr(out=mv, in_=st)
                nc.scalar.activation(out=rstd, in_=mv[:, 1:2], func=AF.Sqrt,
                                     bias=eps_t, scale=1.0)
                nc.vector.reciprocal(out=rstd, in_=rstd)
                nc.vector.tensor_scalar(out=negm, in0=mv[:, 0:1], scalar1=rstd,
                                        scalar2=-1.0, op0=ALU.mult, op1=ALU.mult)
                for lc in range(NCH):
                    lsl = slice(lc * 512, (lc + 1) * 512)
                    pres = psw.tile([128, 512], DT.float32, tag="w")
                    mm(pres, wrt[:, 0, osl], xp[0][:, PAD + lc * 512:PAD + lc * 512 + 512],
                       True, False)
                    mm(pres, wrt[:, 1, osl], xp[1][:, PAD + lc * 512:PAD + lc * 512 + 512],
                       False, False)
                    mm(pres, br[:, osl], ones_row, False, True)
                    tmp = tmpp.tile([128, 512], DT.float32, tag="tmp")
                    nc.scalar.activation(out=tmp, in_=raw[:, lsl],
                                         func=AF.Identity, bias=negm, scale=rstd)
                    nc.vector.tensor_tensor(out=tmp, in0=tmp, in1=pres, op=ALU.add)
                    oc = outp.tile([128, 512], DT.float32, tag="oc")
                    nc.vector.scalar_tensor_tensor(out=oc, in0=tmp, scalar=SLOPE,
                                                   in1=tmp, op0=ALU.mult, op1=ALU.max)
                    nc.sync.dma_start(out=out_d[osl, lsl], in_=oc)
    nc.finalize()
    return nc


def _get_nc():
    global _CACHED_NC
    if _CACHED_NC is None:
        _CACHED_NC = _build()
    return _CACHED_NC


def _pack_weights(inputs):
    f = np.float32
    pack = np.zeros((128, F_PACK), dtype=np.float32)

    def put2(name, w):  # w: [256, ...] -> [128, 2*rest], i-major per partition
        a, b = _SEG[name]
        r = w.reshape(2, 128, -1).transpose(1, 0, 2).reshape(128, -1)
        pack[:, a:b] = r

    put2("w1t", inputs["w1"].astype(f).transpose(1, 2, 0))     # [I,K,O]
    put2("w2t", inputs["w2"].astype(f).transpose(1, 2, 0))
    put2("wqt", inputs["wq"][:, :, 0].astype(f).T)             # [I,O]
    put2("wkt", inputs["wk"][:, :, 0].astype(f).T)
    put2("wvt", inputs["wv"][:, :, 0].astype(f).T)
    put2("wot", inputs["wo"][:, :, 0].astype(f).T)
    put2("wrt", inputs["wr"][:, :, 0].astype(f).T)
    a, b = _SEG["ones_col"]
    pack[:, a:b] = 1.0
    for name in ("bq", "bk", "bv", "bo", "br"):
        a, b = _SEG[name]
        pack[0, a:b] = inputs[name].astype(f)
    a, b = _SEG["ones_row"]
    pack[0, a:b] = 1.0
    return pack.astype(BF16)


def _prep_in_maps(inputs):
    wpack = _pack_weights(inputs)
    x = np.asarray(inputs["x"], dtype=np.float32)
    xpad = np.pad(x, ((0, 0), (0, 0), (PAD, PAD)))              # [B, 256, LP]
    xpad = xpad.reshape(B, 2, 128, LP).transpose(0, 2, 1, 3)    # [B, 128, 2, LP]
    return [{"wpack": wpack, "x": np.ascontiguousarray(xpad[b]).astype(BF16)}
            for b in range(B)]


def run(inputs, trace=False):
    nc = _get_nc()
    in_maps = _prep_in_maps(inputs)
    res = run_bass_kernel_spmd(nc, in_maps, core_ids=list(range(B)), trace=trace)
    out = np.stack([np.asarray(res.results[b]["out"]) for b in range(B)], axis=0)
    return out, res.exec_time_ns


def kernel(**inputs):
    return run(inputs)[0]
